# revision 4
# baseline (speedup 1.0000x reference)
"""MoE routing kernel for Trainium2: softmax over 256 experts + top-8 per token.

Full input: gating_output [131072, 256] f32. Output: (topk_weights f32,
topk_indices int32), both [131072, 8] — matching jax.lax.top_k semantics
(values descending, ties broken by lowest index first).

Strategy: shard tokens row-wise across 8 NeuronCores (16384 tokens each; the
computation is row-local so no communication). Per core, token = p*128 + tt
(partition-major): partition p owns 128 consecutive tokens, processed in
chunks of T subtiles (T consecutive token rows per partition, so each chunk's
input DMA is 128 descriptors of T KiB contiguous). A short-prologue chunk
schedule lets the compute engines start early.

Engine split per chunk:
  DVE : T x InstMax (top-8 raw logits, descending), then T x InstMaxIndex
        (indices; duplicates get ascending distinct indices — matches
        jax.lax.top_k tie rules), plus a tiny reciprocal. This is the
        bottleneck engine: ~721 ns per subtile is the ISA-model floor
        (two full 256-element scans; the input has exact-duplicate and
        <1.5e-5 near-tie rows at the top-8 boundary, so no approximate /
        compressed selection scheme is exact — both scans are required).
  ACT : ONE fused Exp over the whole chunk [128, T*256] (no accumulator),
        plus Exp on the [128, T*8] top-8 logits. Softmax max-subtraction is
        skipped: |x| <= ~5.5 keeps exp well inside f32 range, and softmax is
        shift-invariant.
  Pool: per-token softmax denominators via a log2 tree of strided adds
        (256 -> 128 -> ... -> 1 per token), then the final weights multiply
        exp(top8) * (1/sums). Moving the denominator off ACT removes the
        per-token ACTIVATE + ACCUMULATOR_READ pair (~105us of ACT time)
        that co-bottlenecked with DVE.

Top-k results accumulate in persistent SBUF buffers and flush to DRAM in
quarter-core batches (4 KiB-contiguous runs per partition) so output DMA is
a few large-descriptor transfers instead of thousands of 256 B ones.

Top-8 selection runs on raw logits (softmax is monotone, so same selection),
which avoids f32 ties introduced by exp rounding.
"""

import numpy as np

TOKENS = 131072
EXPERTS = 256
K = 8
N_CORES = 8
TOK_PER_CORE = TOKENS // N_CORES  # 16384
P = 128
TT = TOK_PER_CORE // P  # 128 token rows per partition

# Subtile counts per chunk: short prologue so the first DMA lands fast and
# compute engines spin up early; steady-state 8-subtile (1 MiB) chunks
# (measured tighter DVE stream than 16-subtile chunks: ~1us vs ~3us of
# slack over the intrinsic max8/find_index8 cadence).
CHUNKS = [1, 3, 4] + [8] * 15
assert sum(CHUNKS) == TT

# Flush the persistent output buffers after these many accumulated subtiles.
FLUSH_AT = (32, 64, 96, TT)

_PROGRAM_CACHE = {}


def _build_program():
    import concourse.tile as tile
    from concourse import bacc, mybir

    f32 = mybir.dt.float32
    u32 = mybir.dt.uint32
    Exp = mybir.ActivationFunctionType.Exp

    nc = bacc.Bacc("TRN2", debug=False, num_devices=N_CORES)

    g_dram = nc.dram_tensor(
        "gating", [TOK_PER_CORE, EXPERTS], f32, kind="ExternalInput"
    ).ap()
    w_dram = nc.dram_tensor(
        "weights", [TOK_PER_CORE, K], f32, kind="ExternalOutput"
    ).ap()
    i_dram = nc.dram_tensor(
        "indices", [TOK_PER_CORE, K], u32, kind="ExternalOutput"
    ).ap()

    # token = p*TT + tt: partition-major views
    g_v = g_dram.rearrange("(p tt) e -> p tt e", p=P)  # [128, 128, 256]
    w_v = w_dram.rearrange("(p tt) k -> p tt k", p=P)  # [128, 128, 8]
    i_v = i_dram.rearrange("(p tt) k -> p tt k", p=P)

    with tile.TileContext(nc) as tc:
        with (
            tc.tile_pool(name="gin", bufs=5) as gin_pool,
            tc.tile_pool(name="expbuf", bufs=2) as exp_pool,
            tc.tile_pool(name="outs", bufs=3) as out_pool,
            tc.tile_pool(name="persist", bufs=1) as persist_pool,
        ):
            # persistent per-core result buffers (8 KiB/partition total)
            wbuf = persist_pool.tile([P, TT, K], f32, name="wbuf")
            ibuf = persist_pool.tile([P, TT, K], u32, name="ibuf")

            # self-managed zero bias for the Exp activations: a float bias
            # would become a const AP whose TENSOR_LOAD delays the sync
            # sequencer's first input DMA by ~1us; a Pool-engine memset is
            # off that critical path.
            zbias = persist_pool.tile([P, 1], f32, name="zbias")
            nc.gpsimd.memset(zbias, 0.0)

            ct = 0
            flushed = 0
            fi = 0
            for ci, T in enumerate(CHUNKS):
                gt = gin_pool.tile([P, T * EXPERTS], f32, name=f"gt{ci}", tag="gt")
                nc.sync.dma_start(out=gt, in_=g_v[:, ct : ct + T, :])
                gt3 = gt.rearrange("p (t e) -> p t e", t=T)

                vals = out_pool.tile([P, T, K], f32, name=f"vals{ci}", tag="vals")
                for t in range(T):
                    nc.vector.max(out=vals[:, t, :], in_=gt3[:, t, :])
                for t in range(T):
                    nc.vector.max_index(
                        out=ibuf[:, ct + t, :],
                        in_max=vals[:, t, :],
                        in_values=gt3[:, t, :],
                    )

                # One fused Exp over the whole chunk (ACT), then a log2 tree
                # of strided adds on the Pool engine for per-token sums.
                # This removes the per-token ACTIVATE + ACCUMULATOR_READ pair
                # that made ACT a co-bottleneck with DVE.
                et = exp_pool.tile([P, T * EXPERTS], f32, name=f"et{ci}", tag="et")
                nc.scalar.activation(out=et, in_=gt3, func=Exp, bias=zbias)
                et3 = et.rearrange("p (t e) -> p t e", t=T)

                sc = exp_pool.tile([P, T, 255], f32, name=f"sc{ci}", tag="sc")
                nc.gpsimd.tensor_tensor(
                    out=sc[:, :, 0:128],
                    in0=et3[:, :, 0:128],
                    in1=et3[:, :, 128:256],
                    op=mybir.AluOpType.add,
                )
                off = 0
                w = 64
                while w >= 1:
                    nc.gpsimd.tensor_tensor(
                        out=sc[:, :, off + 2 * w : off + 3 * w],
                        in0=sc[:, :, off : off + w],
                        in1=sc[:, :, off + w : off + 2 * w],
                        op=mybir.AluOpType.add,
                    )
                    off += 2 * w
                    w //= 2
                sums = sc[:, :, 254:255].rearrange(
                    "p t one -> p (t one)"
                )  # [P, T] per-token denominators

                evals = out_pool.tile([P, T, K], f32, name=f"ev{ci}", tag="ev")
                nc.scalar.activation(out=evals, in_=vals, func=Exp, bias=zbias)

                recips = out_pool.tile([P, T], f32, name=f"rec{ci}", tag="rec")
                nc.vector.reciprocal(recips, sums)

                nc.gpsimd.tensor_tensor(
                    out=wbuf[:, ct : ct + T, :],
                    in0=evals,
                    in1=recips.rearrange("p (t one) -> p t one", one=1).to_broadcast(
                        [P, T, K]
                    ),
                    op=mybir.AluOpType.mult,
                )

                ct += T
                if fi < len(FLUSH_AT) and ct >= FLUSH_AT[fi]:
                    nc.sync.dma_start(
                        out=w_v[:, flushed:ct, :], in_=wbuf[:, flushed:ct, :]
                    )
                    nc.sync.dma_start(
                        out=i_v[:, flushed:ct, :], in_=ibuf[:, flushed:ct, :]
                    )
                    flushed = ct
                    fi += 1

    nc.compile()
    return nc


def kernel(**inputs) -> tuple:
    from concourse.bass_utils import run_bass_kernel_spmd

    gating = np.ascontiguousarray(np.asarray(inputs["gating_output"], dtype=np.float32))
    topk = int(np.asarray(inputs.get("topk", K)))
    assert topk == K, f"kernel hardcodes top-{K}, got topk={topk}"
    assert gating.shape == (TOKENS, EXPERTS), gating.shape

    if "nc" not in _PROGRAM_CACHE:
        _PROGRAM_CACHE["nc"] = _build_program()
    nc = _PROGRAM_CACHE["nc"]

    shards = gating.reshape(N_CORES, TOK_PER_CORE, EXPERTS)
    in_maps = [{"gating": shards[c]} for c in range(N_CORES)]
    res = run_bass_kernel_spmd(nc, in_maps, core_ids=list(range(N_CORES)))
    _PROGRAM_CACHE["last_results"] = res

    weights = np.concatenate([r["weights"] for r in res.results], axis=0)
    indices = np.concatenate([r["indices"] for r in res.results], axis=0)
    return weights.astype(np.float32, copy=False), indices.astype(np.int32, copy=False)



# revision 6
# speedup vs baseline: 1.0547x; 1.0547x over previous
"""MoE routing kernel for Trainium2: softmax over 256 experts + top-8 per token.

Full input: gating_output [131072, 256] f32. Output: (topk_weights f32,
topk_indices int32), both [131072, 8] — matching jax.lax.top_k semantics
(values descending, ties broken by lowest index first).

Strategy: shard tokens row-wise across 8 NeuronCores (16384 tokens each; the
computation is row-local so no communication). Per core, token = p*128 + tt
(partition-major): partition p owns 128 consecutive tokens, processed in
chunks of T subtiles (T consecutive token rows per partition, so each chunk's
input DMA is 128 descriptors of T KiB contiguous). A short-prologue chunk
schedule lets the compute engines start early.

Engine split per chunk:
  DVE : T x InstMax (top-8 raw logits, descending), then T x InstMaxIndex
        (indices; duplicates get ascending distinct indices — matches
        jax.lax.top_k tie rules), plus a tiny reciprocal. This is the
        bottleneck engine: ~721 ns per subtile is the ISA-model floor
        (two full 256-element scans; the input has exact-duplicate and
        <1.5e-5 near-tie rows at the top-8 boundary, so no approximate /
        compressed selection scheme is exact — both scans are required).
  ACT : ONE fused Exp over the whole chunk [128, T*256] (no accumulator),
        plus Exp on the [128, T*8] top-8 logits. Softmax max-subtraction is
        skipped: |x| <= ~5.5 keeps exp well inside f32 range, and softmax is
        shift-invariant.
  Pool: per-token softmax denominators via a log2 tree of strided adds
        (256 -> 128 -> ... -> 1 per token), then the final weights multiply
        exp(top8) * (1/sums). Moving the denominator off ACT removes the
        per-token ACTIVATE + ACCUMULATOR_READ pair (~105us of ACT time)
        that co-bottlenecked with DVE.

Top-k results accumulate in persistent SBUF buffers and flush to DRAM in
quarter-core batches (4 KiB-contiguous runs per partition) so output DMA is
a few large-descriptor transfers instead of thousands of 256 B ones.

Top-8 selection runs on raw logits (softmax is monotone, so same selection),
which avoids f32 ties introduced by exp rounding.
"""

import numpy as np

TOKENS = 131072
EXPERTS = 256
K = 8
N_CORES = 8
TOK_PER_CORE = TOKENS // N_CORES  # 16384
P = 128
TT = TOK_PER_CORE // P  # 128 token rows per partition

# Subtile counts per chunk: short prologue so the first DMA lands fast and
# compute engines spin up early; steady-state 8-subtile (1 MiB) chunks
# (measured tighter DVE stream than 16-subtile chunks: ~1us vs ~3us of
# slack over the intrinsic max8/find_index8 cadence).
CHUNKS = [1, 3, 4] + [8] * 15
assert sum(CHUNKS) == TT

# Flush the persistent output buffers after these many accumulated subtiles.
# Denominator reduction, reciprocal, and the weights multiply are batched at
# the same granularity (one "group" = 32 subtiles) to amortize the ~550ns
# fixed cost of every Pool-engine instruction.
FLUSH_AT = (32, 64, 96, TT)
GROUP = 32

_PROGRAM_CACHE = {}


def _build_program():
    import concourse.tile as tile
    from concourse import bacc, mybir

    f32 = mybir.dt.float32
    u32 = mybir.dt.uint32
    Exp = mybir.ActivationFunctionType.Exp

    nc = bacc.Bacc("TRN2", debug=False, num_devices=N_CORES)

    g_dram = nc.dram_tensor(
        "gating", [TOK_PER_CORE, EXPERTS], f32, kind="ExternalInput"
    ).ap()
    w_dram = nc.dram_tensor(
        "weights", [TOK_PER_CORE, K], f32, kind="ExternalOutput"
    ).ap()
    i_dram = nc.dram_tensor(
        "indices", [TOK_PER_CORE, K], u32, kind="ExternalOutput"
    ).ap()

    # token = p*TT + tt: partition-major views
    g_v = g_dram.rearrange("(p tt) e -> p tt e", p=P)  # [128, 128, 256]
    w_v = w_dram.rearrange("(p tt) k -> p tt k", p=P)  # [128, 128, 8]
    i_v = i_dram.rearrange("(p tt) k -> p tt k", p=P)

    with tile.TileContext(nc) as tc:
        with (
            tc.tile_pool(name="gin", bufs=5) as gin_pool,
            tc.tile_pool(name="expbuf", bufs=2) as exp_pool,
            tc.tile_pool(name="outs", bufs=3) as out_pool,
            tc.tile_pool(name="persist", bufs=1) as persist_pool,
        ):
            # persistent per-core result buffers (8 KiB/partition total)
            wbuf = persist_pool.tile([P, TT, K], f32, name="wbuf")
            ibuf = persist_pool.tile([P, TT, K], u32, name="ibuf")

            # self-managed zero bias for the Exp activations: a float bias
            # would become a const AP whose TENSOR_LOAD delays the sync
            # sequencer's first input DMA by ~1us; a Pool-engine memset is
            # off that critical path.
            zbias = persist_pool.tile([P, 1], f32, name="zbias")
            nc.gpsimd.memset(zbias, 0.0)

            # persistent exp(top8) and denominators, written per chunk and
            # consumed per flush group
            evbuf = persist_pool.tile([P, TT, K], f32, name="evbuf")

            ct = 0
            flushed = 0
            fi = 0
            et_group = None
            group_base = 0
            for ci, T in enumerate(CHUNKS):
                if et_group is None:
                    # exp buffer covering a whole 32-subtile flush group
                    et_group = exp_pool.tile(
                        [P, GROUP, EXPERTS], f32, name=f"etg{fi}", tag="etg"
                    )
                    group_base = ct
                gt = gin_pool.tile([P, T * EXPERTS], f32, name=f"gt{ci}", tag="gt")
                nc.sync.dma_start(out=gt, in_=g_v[:, ct : ct + T, :])
                gt3 = gt.rearrange("p (t e) -> p t e", t=T)

                vals = out_pool.tile([P, T, K], f32, name=f"vals{ci}", tag="vals")
                for t in range(T):
                    nc.vector.max(out=vals[:, t, :], in_=gt3[:, t, :])
                for t in range(T):
                    nc.vector.max_index(
                        out=ibuf[:, ct + t, :],
                        in_max=vals[:, t, :],
                        in_values=gt3[:, t, :],
                    )

                # One fused Exp over the whole chunk (ACT) into the group
                # buffer; per-token denominators are reduced once per group.
                go = ct - group_base
                nc.scalar.activation(
                    out=et_group[:, go : go + T, :], in_=gt3, func=Exp, bias=zbias
                )
                nc.scalar.activation(
                    out=evbuf[:, ct : ct + T, :], in_=vals, func=Exp, bias=zbias
                )

                ct += T
                if fi < len(FLUSH_AT) and ct >= FLUSH_AT[fi]:
                    g0 = group_base
                    gn = ct - g0
                    assert gn == GROUP
                    # log2 tree of strided adds on Pool: 256 -> 1 per token.
                    # Level 1 folds the exp buffer onto its own low half
                    # (pure elementwise, in-place-safe); later levels use a
                    # small scratch.
                    nc.gpsimd.tensor_tensor(
                        out=et_group[:, :, 0:128],
                        in0=et_group[:, :, 0:128],
                        in1=et_group[:, :, 128:256],
                        op=mybir.AluOpType.add,
                    )
                    sc = exp_pool.tile([P, gn, 127], f32, name=f"sc{fi}", tag="sc")
                    nc.gpsimd.tensor_tensor(
                        out=sc[:, :, 0:64],
                        in0=et_group[:, :, 0:64],
                        in1=et_group[:, :, 64:128],
                        op=mybir.AluOpType.add,
                    )
                    off = 0
                    w = 32
                    while w >= 1:
                        nc.gpsimd.tensor_tensor(
                            out=sc[:, :, off + 2 * w : off + 3 * w],
                            in0=sc[:, :, off : off + w],
                            in1=sc[:, :, off + w : off + 2 * w],
                            op=mybir.AluOpType.add,
                        )
                        off += 2 * w
                        w //= 2
                    sums = sc[:, :, 126:127].rearrange("p t one -> p (t one)")

                    recips = out_pool.tile([P, gn], f32, name=f"rec{fi}", tag="rec")
                    nc.vector.reciprocal(recips, sums)

                    nc.gpsimd.tensor_tensor(
                        out=wbuf[:, g0:ct, :],
                        in0=evbuf[:, g0:ct, :],
                        in1=recips.rearrange(
                            "p (t one) -> p t one", one=1
                        ).to_broadcast([P, gn, K]),
                        op=mybir.AluOpType.mult,
                    )

                    nc.sync.dma_start(
                        out=w_v[:, flushed:ct, :], in_=wbuf[:, flushed:ct, :]
                    )
                    nc.sync.dma_start(
                        out=i_v[:, flushed:ct, :], in_=ibuf[:, flushed:ct, :]
                    )
                    flushed = ct
                    fi += 1
                    et_group = None

    nc.compile()
    return nc


def kernel(**inputs) -> tuple:
    from concourse.bass_utils import run_bass_kernel_spmd

    gating = np.ascontiguousarray(np.asarray(inputs["gating_output"], dtype=np.float32))
    topk = int(np.asarray(inputs.get("topk", K)))
    assert topk == K, f"kernel hardcodes top-{K}, got topk={topk}"
    assert gating.shape == (TOKENS, EXPERTS), gating.shape

    if "nc" not in _PROGRAM_CACHE:
        _PROGRAM_CACHE["nc"] = _build_program()
    nc = _PROGRAM_CACHE["nc"]

    shards = gating.reshape(N_CORES, TOK_PER_CORE, EXPERTS)
    in_maps = [{"gating": shards[c]} for c in range(N_CORES)]
    res = run_bass_kernel_spmd(nc, in_maps, core_ids=list(range(N_CORES)))
    _PROGRAM_CACHE["last_results"] = res

    weights = np.concatenate([r["weights"] for r in res.results], axis=0)
    indices = np.concatenate([r["indices"] for r in res.results], axis=0)
    return weights.astype(np.float32, copy=False), indices.astype(np.int32, copy=False)

